# revision 28
# baseline (speedup 1.0000x reference)
"""Trainium2 Bass kernel for the GroupPointEncoder problem.

Strategy
--------
The reference output has huge redundancy:
  * the (K_ONE2MANY+1)=6x tile along M means only N=500 unique points per
    (group, batch) slice need the sin/cos embedding + MLP,
  * out[..., 0:256] is a plain broadcast of query_pos for every slice.

Sharding: data-parallel over the 32 (group, batch) slices -> 4 slices per
NeuronCore, params replicated. Per slice the device computes
    c_norm = (point + raw_noise * std(labels) - lo) / (hi - lo)       [3, 500]
    posT   = sin(S[f] * c_norm[axis] + phase[f])                      [384, 500]
    hT     = relu(W1 @ pos + b1)                                      [512, 500]
    qT     = W2 @ h + b2 + label_embed[labels]  (gather via one-hot)  [256, 500]
then transposes q/c_norm to point-major via the PE and streams the
[3000, 512] output slice (query_pos half + 6x tiled q half) with large DMAs.

All feature-major GEMMs keep the contraction on SBUF partitions, so no
transposes are needed anywhere except the final point-major writeback.
"""

import math
import os

import numpy as np

NUM_FEATS = 128
NUM_GROUP = 4
KP1 = 6                      # K_ONE2MANY + 1
TEMPERATURE = 10000.0
SMALL_CATS = (0, 6, 7, 8, 9)
BS = 8
N = 500
M = N * KP1                  # 3000
NCORES = 8
SLICES = NUM_GROUP * BS // NCORES   # 4 (group, batch) slices per core
PCH = 125                    # point-chunk for point-major layouts (500 = 4*125)
QCH = 24                     # query_pos row chunks (3000 = 24*125)
AX_ORDER = (1, 0, 2)         # pos embed concat order: (y, x, z)

_cache = {}


def _mm_dt_name():
    return os.environ.get("GPE_MM_DT", "float32r")


def _host_constants():
    if "consts" in _cache:
        return _cache["consts"]
    f = np.arange(NUM_FEATS, dtype=np.float64)
    dim_t = TEMPERATURE ** (2.0 * np.floor(f / 2.0) / NUM_FEATS)
    S = (2.0 * math.pi / dim_t).astype(np.float32)          # [128]
    # Embed matmul lhsT [4, 3*128]: rows 0-2 select the coordinate row and
    # scale by S; row 3 (paired with an all-ones rhs row) adds the per-channel
    # sin/cos phase.  Range reduction into ScalarE's [-pi, pi] sin domain
    # happens afterwards on the DVE (cast-based round + one fix-up round).
    phase = np.where(f.astype(np.int64) % 2 == 0, 0.0, math.pi / 2.0)
    selS = np.zeros((4, 3 * NUM_FEATS), dtype=np.float32)
    for a, ax in enumerate(AX_ORDER):
        selS[ax, a * NUM_FEATS:(a + 1) * NUM_FEATS] = S
        selS[3, a * NUM_FEATS:(a + 1) * NUM_FEATS] = phase
    iota10 = np.arange(10, dtype=np.float32).reshape(10, 1)
    ones10 = np.ones((1, 10), dtype=np.float32)
    stdlut = np.where(np.isin(np.arange(10), SMALL_CATS), 2.0, 4.0)
    stdlut = np.repeat(stdlut.astype(np.float32).reshape(10, 1), 3, axis=1)  # [10,3]
    ident = np.eye(128, dtype=np.float32)
    _cache["consts"] = dict(selS=selS, iota10=iota10,
                            ones10=ones10, stdlut=stdlut, ident=ident)
    return _cache["consts"]


def _raw_noise():
    # Input-independent constant: jax.random.normal(key(42), ...) computed on
    # the CPU backend to match the reference bit-for-bit.
    if "noise" not in _cache:
        import jax
        import jax.numpy as jnp
        with jax.default_device(jax.devices("cpu")[0]):
            nz = jax.random.normal(jax.random.key(42),
                                   (NUM_GROUP, BS, N, 3), dtype=jnp.float32)
            _cache["noise"] = np.asarray(nz)
    return _cache["noise"]


def _build_program(mm_dt_name):
    from contextlib import ExitStack

    import concourse.bass as bass
    import concourse.tile as tile
    from concourse import bacc, mybir

    f32 = mybir.dt.float32
    i32 = mybir.dt.int32
    mm_dt = getattr(mybir.dt, mm_dt_name)
    A = mybir.ActivationFunctionType
    OP = mybir.AluOpType

    nc = bacc.Bacc("TRN2", target_bir_lowering=False, debug=False,
                   num_devices=NCORES)

    # ---- DRAM I/O ----
    pts_d = nc.dram_tensor("pts", [SLICES, N, 3], f32, kind="ExternalInput")
    nz_d = nc.dram_tensor("nz", [SLICES, N, 3], f32, kind="ExternalInput")
    labf_d = nc.dram_tensor("labf", [SLICES, N], f32, kind="ExternalInput")
    pcr_d = nc.dram_tensor("pcr", [3, 2], f32, kind="ExternalInput")
    le_d = nc.dram_tensor("le", [10, 2 * NUM_FEATS], f32, kind="ExternalInput")
    w1t_d = nc.dram_tensor("w1t", [3 * NUM_FEATS, 4 * NUM_FEATS], mm_dt,
                           kind="ExternalInput")
    w2t_d = nc.dram_tensor("w2t", [4 * NUM_FEATS, 2 * NUM_FEATS], mm_dt,
                           kind="ExternalInput")
    b1_d = nc.dram_tensor("b1t", [NUM_FEATS, 4], f32, kind="ExternalInput")
    b2_d = nc.dram_tensor("b2t", [NUM_FEATS, 2], f32, kind="ExternalInput")
    qp_d = nc.dram_tensor("qp", [M, 2 * NUM_FEATS], f32, kind="ExternalInput")
    selS_d = nc.dram_tensor("selS", [4, 3 * NUM_FEATS], f32, kind="ExternalInput")
    iota_d = nc.dram_tensor("iota10", [10, 1], f32, kind="ExternalInput")
    ones_d = nc.dram_tensor("ones10", [1, 10], f32, kind="ExternalInput")
    stdl_d = nc.dram_tensor("stdlut", [10, 3], f32, kind="ExternalInput")
    id_d = nc.dram_tensor("ident", [128, 128], f32, kind="ExternalInput")
    ones_n_d = nc.dram_tensor("ones500", [1, N], f32, kind="ExternalInput")

    out_d = nc.dram_tensor("out_sh", [SLICES, M, 4 * NUM_FEATS], f32,
                           kind="ExternalOutput")
    crd_d = nc.dram_tensor("coords_sh", [SLICES, M, 3], f32,
                           kind="ExternalOutput")

    out_ap = out_d.ap()
    crd_ap = crd_d.ap()

    with tile.TileContext(nc) as tc, ExitStack() as ctx:
        cpool = ctx.enter_context(tc.tile_pool(name="const", bufs=1))
        wpool = ctx.enter_context(tc.tile_pool(name="work", bufs=2))
        ppool = ctx.enter_context(tc.tile_pool(name="ps", bufs=8, space="PSUM"))

        # ---- constants into SBUF ----
        def cload(name, shape, src_ap):
            t = cpool.tile(shape, f32, tag=name)
            nc.gpsimd.dma_start(t[:], src_ap)
            return t

        selS_s = cload("selS", [4, 3 * NUM_FEATS], selS_d.ap())
        iota_s = cload("iota10", [10, 1], iota_d.ap())
        ones_s = cload("ones10", [1, 10], ones_d.ap())
        stdl_s = cload("stdlut", [10, 3], stdl_d.ap())
        id_s = cload("ident", [128, 128], id_d.ap())
        le_s = cload("le", [10, 2 * NUM_FEATS], le_d.ap())
        b1_s = cload("b1t", [NUM_FEATS, 4], b1_d.ap())
        b2_s = cload("b2t", [NUM_FEATS, 2], b2_d.ap())
        pcr_s = cload("pcr", [3, 2], pcr_d.ap())
        w1t_s = cpool.tile([128, 3 * 512], mm_dt, tag="w1t")
        nc.gpsimd.dma_start(
            w1t_s[:].rearrange("p (k o) -> p k o", o=512),
            w1t_d.ap().rearrange("(k p) o -> p k o", p=128))
        w2t_s = cpool.tile([128, 4 * 256], mm_dt, tag="w2t")
        nc.gpsimd.dma_start(
            w2t_s[:].rearrange("p (k o) -> p k o", o=256),
            w2t_d.ap().rearrange("(k p) o -> p k o", p=128))
        qp_s = cpool.tile([PCH, QCH * 256], f32, tag="qp")
        nc.gpsimd.dma_start(
            qp_s[:].rearrange("p (t c) -> p t c", c=256),
            qp_d.ap().rearrange("(t p) c -> p t c", p=PCH))

        # normalization constants from pc_range: -lo and 1/(hi-lo), [3,1]
        neglo_s = cpool.tile([3, 1], f32, tag="neglo")
        inv_s = cpool.tile([3, 1], f32, tag="inv")
        nc.vector.tensor_scalar_mul(neglo_s[:], pcr_s[:, 0:1], -1.0)
        nc.vector.tensor_sub(inv_s[:], pcr_s[:, 1:2], pcr_s[:, 0:1])
        nc.vector.reciprocal(inv_s[:], inv_s[:])

        for s in range(SLICES):
            # ---- inputs ----
            labf = wpool.tile([1, N], f32, tag="labf")
            nc.gpsimd.dma_start(labf[:], labf_d.ap()[s:s + 1, :])
            pt_t = wpool.tile([3, N], f32, tag="pt_t")
            nc.gpsimd.dma_start(pt_t[:], pts_d.ap()[s].rearrange("n c -> c n"))
            nz_t = wpool.tile([3, N], f32, tag="nz_t")
            nc.gpsimd.dma_start(nz_t[:], nz_d.ap()[s].rearrange("n c -> c n"))

            # ---- one-hot(labels): broadcast labels to 10 partitions via PE,
            # then compare against iota ----
            ps_oh = ppool.tile([10, N], f32, tag="ps")
            nc.tensor.matmul(ps_oh[:], ones_s[:], labf[:], start=True, stop=True)
            oh = wpool.tile([10, N], f32, tag="oh")
            nc.vector.tensor_scalar(oh[:], ps_oh[:], iota_s[:, 0:1], None,
                                    op0=OP.is_equal)

            # ---- std lookup + normalized coords; row 3 = ones (phase row
            # of the embed matmul) ----
            ps_std = ppool.tile([3, N], f32, tag="ps")
            nc.tensor.matmul(ps_std[:], stdl_s[:], oh[:], start=True, stop=True)
            cn = wpool.tile([4, N], f32, tag="cn")
            nc.gpsimd.dma_start(cn[3:4, :], ones_n_d.ap())
            nc.vector.tensor_tensor(cn[0:3, :], nz_t[:], ps_std[:], OP.mult)
            nc.vector.scalar_tensor_tensor(cn[0:3, :], pt_t[:], neglo_s[:, 0:1],
                                           cn[0:3, :], OP.add, OP.add)
            nc.vector.tensor_scalar_mul(cn[0:3, :], cn[0:3, :], inv_s[:, 0:1])

            # ---- sin/cos positional embedding, feature-major [384, N] ----
            posT = wpool.tile([128, 3 * N], mm_dt, tag="posT")
            for a in range(3):
                ps_e = ppool.tile([128, N], f32, tag="ps")
                nc.tensor.matmul(ps_e[:], selS_s[:, a * 128:(a + 1) * 128],
                                 cn[:], start=True, stop=True)
                # range reduction: k = int(x/2pi + 0.5) via the DVE output
                # cast (trunc or round-to-nearest both fine), y = x - 2pi*k,
                # then one fix-up round (+2pi where y < -pi) makes the result
                # land in [-pi, pi) regardless of the hardware rounding mode.
                emb_k = wpool.tile([128, N], i32, tag="emb_k")
                nc.vector.tensor_scalar(emb_k[:], ps_e[:],
                                        1.0 / (2.0 * math.pi), 0.5,
                                        op0=OP.mult, op1=OP.add)
                nc.vector.scalar_tensor_tensor(ps_e[:], emb_k[:],
                                               -2.0 * math.pi, ps_e[:],
                                               OP.mult, OP.add)
                emb_t = wpool.tile([128, N], f32, tag="emb_t")
                nc.vector.tensor_scalar(emb_t[:], ps_e[:], -math.pi,
                                        2.0 * math.pi, op0=OP.is_lt, op1=OP.mult)
                nc.vector.tensor_tensor(ps_e[:], ps_e[:], emb_t[:], OP.add)
                nc.scalar.activation(posT[:, a * N:(a + 1) * N], ps_e[:],
                                     A.Sin)

            # ---- GEMM1: hT = relu(W1 @ pos + b1), [512, N] as 4 chunks ----
            hT = wpool.tile([128, 4 * N], mm_dt, tag="hT")
            for m in range(4):
                ps1 = ppool.tile([128, N], f32, tag="ps")
                for k in range(3):
                    nc.tensor.matmul(
                        ps1[:],
                        w1t_s[:, k * 512 + m * 128:k * 512 + (m + 1) * 128],
                        posT[:, k * N:(k + 1) * N],
                        start=(k == 0), stop=(k == 2))
                nc.scalar.activation(hT[:, m * N:(m + 1) * N], ps1[:],
                                     A.Relu, bias=b1_s[:, m:m + 1])

            # ---- GEMM2 + label-embed gather: qT = W2 @ h + b2 + LE^T @ onehot ----
            qT = wpool.tile([128, 2 * N], f32, tag="qT")
            for mo in range(2):
                ps2 = ppool.tile([128, N], f32, tag="ps")
                for k in range(4):
                    nc.tensor.matmul(
                        ps2[:],
                        w2t_s[:, k * 256 + mo * 128:k * 256 + (mo + 1) * 128],
                        hT[:, k * N:(k + 1) * N],
                        start=(k == 0), stop=False)
                nc.tensor.matmul(ps2[:], le_s[:, mo * 128:(mo + 1) * 128],
                                 oh[:], start=False, stop=True)
                nc.scalar.activation(qT[:, mo * N:(mo + 1) * N], ps2[:],
                                     A.Identity, bias=b2_s[:, mo:mo + 1])

            # ---- transpose q to point-major [125-chunks, 256] ----
            q_nat = wpool.tile([PCH, 4 * 256], f32, tag="q_nat")
            for j in range(4):
                ps_t = ppool.tile([PCH, 256], f32, tag="ps")
                for mo in range(2):
                    nc.tensor.transpose(
                        ps_t[:, mo * 128:(mo + 1) * 128],
                        qT[:, mo * N + j * PCH:mo * N + (j + 1) * PCH],
                        id_s[:])
                nc.vector.tensor_copy(q_nat[:, j * 256:(j + 1) * 256], ps_t[:])

            # ---- transpose coords to point-major [125-chunks, 3] ----
            ps_c = ppool.tile([PCH, 12], f32, tag="ps")
            for j in range(4):
                nc.tensor.transpose(ps_c[:, j * 3:(j + 1) * 3],
                                    cn[0:3, j * PCH:(j + 1) * PCH],
                                    id_s[0:3, 0:3])
            cn_nat = wpool.tile([PCH, 12], f32, tag="cn_nat")
            nc.vector.tensor_copy(cn_nat[:], ps_c[:])

            # ---- output writes ----
            # query_pos half: one 3 MB DMA per slice
            nc.sync.dma_start(
                out_ap[s, :, 0:256].rearrange("(t p) c -> p t c", p=PCH),
                qp_s[:].rearrange("p (t c) -> p t c", c=256))
            # q half + coords: 6x tiled
            for r in range(KP1):
                nc.sync.dma_start(
                    out_ap[s, r * N:(r + 1) * N, 256:512]
                        .rearrange("(k p) c -> p k c", p=PCH),
                    q_nat[:].rearrange("p (k c) -> p k c", c=256))
                nc.sync.dma_start(
                    crd_ap[s, r * N:(r + 1) * N, :]
                        .rearrange("(k p) c -> p k c", p=PCH),
                    cn_nat[:].rearrange("p (k c) -> p k c", c=3))

    nc.compile()
    return nc


def _get_program():
    key = ("prog", _mm_dt_name())
    if key not in _cache:
        _cache[key] = _build_program(_mm_dt_name())
    return _cache[key]


def make_in_maps(point_coord, labels, pc_range, label_embed, W1, b1, W2, b2,
                 query_pos):
    """Build the per-core input maps (host-side sharding + constant prep)."""
    point_coord = np.ascontiguousarray(point_coord, dtype=np.float32)
    pc_range = np.asarray(pc_range, dtype=np.float32)
    label_embed = np.ascontiguousarray(label_embed, dtype=np.float32)
    W1 = np.asarray(W1, dtype=np.float32)
    W2 = np.asarray(W2, dtype=np.float32)
    b1 = np.asarray(b1, dtype=np.float32)
    b2 = np.asarray(b2, dtype=np.float32)
    query_pos = np.ascontiguousarray(query_pos, dtype=np.float32)
    labf = np.asarray(labels).astype(np.float32)

    consts = _host_constants()
    noise = _raw_noise()

    shared = dict(
        pcr=np.ascontiguousarray(np.stack([pc_range[:3], pc_range[3:]], axis=1)),
        le=label_embed,
        w1t=np.ascontiguousarray(W1.T),
        w2t=np.ascontiguousarray(W2.T),
        b1t=np.ascontiguousarray(b1.reshape(4, 128).T),
        b2t=np.ascontiguousarray(b2.reshape(2, 128).T),
        qp=query_pos,
        selS=consts["selS"], iota10=consts["iota10"],
        ones10=consts["ones10"], stdlut=consts["stdlut"], ident=consts["ident"],
        ones500=np.ones((1, N), dtype=np.float32),
    )

    in_maps = []
    for c in range(NCORES):
        gbs = [c * SLICES + i for i in range(SLICES)]
        pts = np.stack([point_coord[gb % BS] for gb in gbs])
        nz = np.stack([np.zeros((N, 3), np.float32) if gb // BS == 0
                       else noise[gb // BS, gb % BS] for gb in gbs])
        lf = np.stack([labf[gb % BS] for gb in gbs])
        in_maps.append(dict(pts=pts, nz=np.ascontiguousarray(nz),
                            labf=np.ascontiguousarray(lf), **shared))
    return in_maps


def assemble_outputs(results, labels):
    """Stitch per-core shards into the full reference-shaped outputs."""
    out = np.concatenate([r["out_sh"] for r in results], axis=0)
    coords = np.concatenate([r["coords_sh"] for r in results], axis=0)
    coords = coords.reshape(NUM_GROUP, BS, M, 3)
    labels = np.asarray(labels)
    lab = np.tile(labels[None, :, :], (NUM_GROUP, 1, KP1))
    return out, coords, lab


def kernel(point_coord, labels, pc_range, label_embed, W1, b1, W2, b2,
           query_pos):
    from concourse import bass_utils

    nc = _get_program()
    in_maps = make_in_maps(point_coord, labels, pc_range, label_embed,
                           W1, b1, W2, b2, query_pos)
    res = bass_utils.run_bass_kernel_spmd(nc, in_maps,
                                          core_ids=list(range(NCORES)))
    return assemble_outputs(res.results, labels)
